# revision 4
# baseline (speedup 1.0000x reference)
"""Multi-head causal attention (GPT-2 style) on 8 TRN2 NeuronCores — v3.

Sharding: core i handles batch i//2 and head-group i%2 (8 of 16 heads).
Partial output projections are summed on the host along with the
exactly-factored bias terms (bq on-device; bk softmax-invariant, dropped;
bv/bp commute through attention: y += bv @ Wp + bp on host).

v3 over v2:
  - software pipelining: qk_proj(mc+1) groups are interleaved INTO the
    attention kt-loop of pair mc, and yproj tiles into pair 3, so the PE
    instruction stream never waits on ACT exp latency.
  - diagonal tiles extended from N=128 to N=256 with a zero-extended mask
    (f32r matmuls run 4x slower below N=256; masked-out exp cols add 0).
  - v_sb / ctxT double-buffered so loop iterations overlap (the V
    projection of iter i+1 no longer waits for the last ctx matmul of i).
  - head-pair row tiling: the two heads of an mc-slice live on PE row
    groups 0-63/64-127 (K=DH=64); their score matmuls are emitted
    back-to-back and run concurrently.
All matmuls float32r (~1e-4 rel), fp32 PSUM accumulate.
"""
import numpy as np

import concourse.bacc as bacc
import concourse.mybir as mybir
import concourse.tile as tile
from concourse.bass_utils import run_bass_kernel_spmd

B, T, C, H, DH = 4, 1024, 1024, 16, 64
P = 128
CS = 512
F32 = mybir.dt.float32
F32R = mybir.dt.float32r
MM_DTYPE = F32R
AF = mybir.ActivationFunctionType
N_CORES = 8


def build_nc(loop_n=None, mm_dtype=None, proj_bufs=2, interleave=True,
             dbuf=True):
    MMD = mm_dtype or MM_DTYPE
    nc = bacc.Bacc("TRN2", target_bir_lowering=False, debug=False,
                   num_devices=N_CORES)
    xT = nc.dram_tensor("xT", [C, T], MMD, kind="ExternalInput")
    wq = nc.dram_tensor("wq", [C, CS], MMD, kind="ExternalInput")
    wk = nc.dram_tensor("wk", [C, CS], MMD, kind="ExternalInput")
    wv = nc.dram_tensor("wv", [C, CS], MMD, kind="ExternalInput")
    wp = nc.dram_tensor("wp", [CS, C], MMD, kind="ExternalInput")
    bq = nc.dram_tensor("bq", [P, 4], F32, kind="ExternalInput")
    # mask[:, 0:128] = zeros, mask[:, 128:256] = causal tril block
    mask = nc.dram_tensor("mask", [P, 2 * P], F32, kind="ExternalInput")
    ones = nc.dram_tensor("ones", [P, 64], MMD, kind="ExternalInput")
    y = nc.dram_tensor("y", [T, C], F32, kind="ExternalOutput")

    with tile.TileContext(nc) as tc:
        with (
            tc.tile_pool(name="big", bufs=1) as big,
            tc.tile_pool(name="dpool", bufs=2 if dbuf else 1) as dpool,
            tc.tile_pool(name="es_pool", bufs=4) as es_pool,
            tc.tile_pool(name="y_pool", bufs=3) as y_pool,
            tc.tile_pool(name="small", bufs=2) as small,
            tc.tile_pool(name="proj_ps", bufs=proj_bufs, space="PSUM") as proj_ps,
            tc.tile_pool(name="sc_ps", bufs=4, space="PSUM") as sc_ps,
            tc.tile_pool(name="ctx_ps", bufs=1, space="PSUM") as ctx_ps,
        ):
            from contextlib import ExitStack
            _ls = ExitStack()
            if loop_n:
                _ls.enter_context(tc.For_i(0, loop_n, 1))
            xT_sb = big.tile([P, 8, T], MMD)
            wq_sb = big.tile([P, 8, CS], MMD)
            wk_sb = big.tile([P, 8, CS], MMD)
            wv_sb = big.tile([P, 8, CS], MMD)
            wp_sb = big.tile([P, 4, C], MMD)
            bq_sb = big.tile([P, 4], F32)
            mask_sb = big.tile([P, 2 * P], F32)
            qT_sb = big.tile([P, 4, 2, 512], MMD)
            kT_sb = big.tile([P, 4, 2, 512], MMD)
            v_sb = dpool.tile([P, 8, 8, 65], MMD, tag="v")
            ctxT_sb = big.tile([P, 4, T], MMD)

            nc.sync.dma_start(out=bq_sb, in_=bq.ap())
            nc.sync.dma_start(out=mask_sb, in_=mask.ap())
            xT_r = xT.ap().rearrange("(c p) t -> p c t", p=P)
            wq_r = wq.ap().rearrange("(c p) n -> p c n", p=P)
            wk_r = wk.ap().rearrange("(c p) n -> p c n", p=P)
            wv_r = wv.ap().rearrange("(c p) n -> p c n", p=P)
            for c in range(8):
                nc.sync.dma_start(out=xT_sb[:, c, :], in_=xT_r[:, c, :])
                nc.sync.dma_start(out=wv_sb[:, c, :], in_=wv_r[:, c, :])
                nc.sync.dma_start(out=wq_sb[:, c, :], in_=wq_r[:, c, :])
                nc.sync.dma_start(out=wk_sb[:, c, :], in_=wk_r[:, c, :])
            wp_r = wp.ap().rearrange("(k p) n -> p k n", p=P)
            for kc in range(4):
                nc.sync.dma_start(out=wp_sb[:, kc, :], in_=wp_r[:, kc, :])

            # ---- V natural [T, 512] + ones column per head ----
            nc.sync.dma_start(out=v_sb[:, :, :, 64],
                              in_=ones.ap().rearrange("p (a b) -> p a b", a=8))
            for tt in range(8):
                ps = proj_ps.tile([P, 512], F32, tag="proj")
                for c in range(8):
                    nc.tensor.matmul(
                        ps, xT_sb[:, c, tt * P:(tt + 1) * P], wv_sb[:, c, :],
                        start=(c == 0), stop=(c == 7))
                nc.vector.tensor_copy(
                    v_sb[:, tt, :, 0:64],
                    ps.rearrange("p (h d) -> p h d", h=8))

            def qk_group(mc, which, tc2):
                """One projection group: 8 matmuls + PSUM->SBUF copy."""
                wsb, outsb, is_q = ((wq_sb, qT_sb, True) if which == 0
                                    else (wk_sb, kT_sb, False))
                ps = proj_ps.tile([P, 512], F32, tag="proj", name="qkps")
                for c in range(8):
                    nc.tensor.matmul(
                        ps, wsb[:, c, mc * P:(mc + 1) * P],
                        xT_sb[:, c, tc2 * 512:(tc2 + 1) * 512],
                        start=(c == 0), stop=(c == 7))
                dst = outsb[:, mc, tc2, :]
                if is_q:
                    nc.vector.tensor_add(
                        dst, ps, bq_sb[:, mc:mc + 1].broadcast_to([P, 512]))
                else:
                    nc.vector.tensor_copy(dst, ps)

            def qk_proj_groups(mc):
                for which in (1, 0):          # K first: scores need kT early
                    for tc2 in range(2):
                        yield lambda w=which, t=tc2: qk_group(mc, w, t)

            def yproj_group(tt, nk):
                ps = proj_ps.tile([P, 512], F32, tag="proj", name="yps")
                for kc in range(4):
                    nc.tensor.matmul(
                        ps, ctxT_sb[:, kc, tt * P:(tt + 1) * P],
                        wp_sb[:, kc, nk * 512:(nk + 1) * 512],
                        start=(kc == 0), stop=(kc == 3))
                ysb = y_pool.tile([P, 512], F32, tag="y", name="ysb")
                nc.vector.tensor_copy(ysb, ps)
                nc.sync.dma_start(
                    out=y.ap()[tt * P:(tt + 1) * P, nk * 512:(nk + 1) * 512],
                    in_=ysb)

            def norm_write(h, qc, cps):
                hp = (h % 2) * 64
                mc = h // 2
                recr = small.tile([1, 512], F32, tag="recr", name="recr")
                nc.vector.reciprocal(recr, cps[64:65, :])
                recb = small.tile([64, 512], F32, tag="recb", name="recb")
                nc.gpsimd.partition_broadcast(recb, recr)
                nc.vector.tensor_mul(
                    ctxT_sb[hp:hp + 64, mc, qc * 512:(qc + 1) * 512],
                    cps[0:64, :], recb)

            def attention_steps(mc, qc, cps):
                """Per-kt emission steps for head pair mc, q-chunk qc."""
                hA, hB = 2 * mc, 2 * mc + 1
                nkt = 4 if qc == 0 else 8

                def step(kt):
                    if qc == 0:
                        r0, diag = kt * P, True
                    elif kt < 4:
                        r0, diag = 0, False
                    else:
                        r0, diag = (kt - 4) * P, True
                    # diagonal tile at N=128 -> extend to N=256 with the
                    # zero-extended mask (f32r needs N>=256 for full rate)
                    ext = diag and r0 == 384
                    if ext:
                        r0 = 256
                    for h, hp in ((hA, 0), (hB, 64)):
                        sps = sc_ps.tile([P, 512], F32, tag="sc",
                                         name=f"sps{h % 2}")
                        nc.tensor.matmul(
                            sps[:, r0:],
                            kT_sb[hp:hp + 64, mc, kt // 4,
                                  (kt % 4) * P:(kt % 4 + 1) * P],
                            qT_sb[hp:hp + 64, mc, qc, r0:],
                            start=True, stop=True)
                        es = es_pool.tile([P, 512], MMD, tag="es",
                                          name=f"es{h % 2}")
                        nc.scalar.activation(es[:, r0:], sps[:, r0:], AF.Exp)
                        if diag:
                            if ext:
                                nc.vector.tensor_mul(es[:, r0:r0 + 2 * P],
                                                     es[:, r0:r0 + 2 * P],
                                                     mask_sb)
                            else:
                                nc.vector.tensor_mul(es[:, r0:r0 + P],
                                                     es[:, r0:r0 + P],
                                                     mask_sb[:, P:])
                        nc.tensor.matmul(cps[h][:, r0:],
                                         v_sb[:, kt, h, :],
                                         es[:, r0:],
                                         start=(kt == 0),
                                         stop=(kt == nkt - 1))

                for kt in range(nkt):
                    yield lambda k=kt: step(k)

            def run_interleaved(steps, fillers, period):
                """Emit attention steps with one filler group every
                `period` steps (fillers keep the PE busy during exp)."""
                i = 0
                fillers = list(fillers)
                fi = 0
                for s in steps:
                    s()
                    i += 1
                    if interleave and fi < len(fillers) and i % period == 0:
                        fillers[fi]()
                        fi += 1
                while fi < len(fillers):
                    fillers[fi]()
                    fi += 1

            # ---- main schedule ----
            for g in qk_proj_groups(0):
                g()
            for mc in range(4):
                hA, hB = 2 * mc, 2 * mc + 1
                cps = {hA: ctx_ps.tile([65, 512], F32, tag="ctxA", name="cpsA"),
                       hB: ctx_ps.tile([65, 512], F32, tag="ctxB", name="cpsB")}
                if mc < 3:
                    allf = list(qk_proj_groups(mc + 1))
                    fill0, fill1 = allf[:2], allf[2:]
                else:
                    fill0 = []
                    fill1 = [lambda t=tt, n=nk: yproj_group(t, n)
                             for tt in range(4) for nk in range(2)]
                run_interleaved(attention_steps(mc, 0, cps), fill0, 2)
                for h in (hA, hB):
                    norm_write(h, 0, cps[h])
                cps = {hA: ctx_ps.tile([65, 512], F32, tag="ctxA", name="cpsA"),
                       hB: ctx_ps.tile([65, 512], F32, tag="ctxB", name="cpsB")}
                run_interleaved(attention_steps(mc, 1, cps), fill1, 2)
                for h in (hA, hB):
                    norm_write(h, 1, cps[h])
            for tt in range(4, 8):
                for nk in range(2):
                    yproj_group(tt, nk)
            _ls.close()
    nc.compile()
    return nc


_NC = None


def _get_nc():
    global _NC
    if _NC is None:
        _NC = build_nc()
    return _NC


def make_in_maps(x, Wq, bq, Wk, Wv, Wp, mm_dtype=None):
    """Per-core input dicts."""
    import ml_dtypes
    MMD = mm_dtype or MM_DTYPE
    cvt = ((lambda a: np.ascontiguousarray(a).astype(ml_dtypes.bfloat16))
           if MMD == mybir.dt.bfloat16 else np.ascontiguousarray)
    tril = (np.arange(P)[None, :] >= np.arange(P)[:, None]).astype(np.float32)
    masks = np.concatenate([np.zeros((P, P), np.float32), tril], axis=1)
    in_maps = []
    for core in range(N_CORES):
        b = core // 2
        g = core % 2
        cs = slice(g * CS, (g + 1) * CS)
        in_maps.append(dict(
            xT=cvt(x[b].T),
            wq=cvt(Wq[:, cs] * np.float32(0.125)),
            wk=cvt(Wk[:, cs]),
            wv=cvt(Wv[:, cs]),
            wp=cvt(Wp[cs, :]),
            bq=np.ascontiguousarray((bq[cs] * np.float32(0.125))
                                    .reshape(4, P).T),
            mask=masks,
            ones=cvt(np.ones((P, 64), np.float32)),
        ))
    return in_maps


def combine(parts, Wq, bv, Wp, bp):
    """parts: list of 8 per-core partial y arrays -> full [B, T, C] output."""
    out = np.stack([parts[2 * b] + parts[2 * b + 1] for b in range(B)])
    out += (bv @ Wp + bp)[None, None, :]
    return out.astype(np.float32)


def kernel(**inputs):
    x = np.asarray(inputs["x"], np.float32)
    Wq = np.asarray(inputs["Wq"], np.float32)
    bq = np.asarray(inputs["bq"], np.float32)
    Wk = np.asarray(inputs["Wk"], np.float32)
    Wv = np.asarray(inputs["Wv"], np.float32)
    Wp = np.asarray(inputs["Wp"], np.float32)
    bv = np.asarray(inputs["bv"], np.float32)
    bp = np.asarray(inputs["bp"], np.float32)
    # bk intentionally unused: softmax cancels it exactly.

    nc = _get_nc()
    in_maps = make_in_maps(x, Wq, bq, Wk, Wv, Wp)
    res = run_bass_kernel_spmd(nc, in_maps, core_ids=list(range(N_CORES)))
    parts = [res.results[c]["y"] for c in range(N_CORES)]
    return combine(parts, Wq, bv, Wp, bp)
